# revision 20
# baseline (speedup 1.0000x reference)
"""Trainium2 Bass kernel for nn_DQNAgent_53154515255692 (topk_masking).

For each batch b and every unordered node pair {i,j} of O[b] (16, 256, 64):
    feats = [Oi, Oj, |Oi-Oj|, Oi*Oj]            (B, M, 256), M = 32640
    Q     = relu(feats @ W1 + b1) @ W2 + b2     (B, M)
    mask  = one-hot top-2 of Q per row          (B, M) bool
Returns (Q, mask) exactly like the reference.

Sharding: data-parallel over batch - 2 batches per NeuronCore on 8 cores,
tiny MLP weights replicated. See build_nc for the device algorithm: fp16
matmul path (1 cycle/column on the PE, measured safe: |dQ| ~ 1.6e-3 vs a
6.9e-3 minimum top-2 gap), circular 130-column pair windows so both rows of
a pair-block share every tensor op at full 128-partition occupancy, W1
split into four 64x64 blocks (W1a/W1b per-node terms via matmuls with
wrap-region weight/bias swap; |d|/p by DVE/GPSIMD elementwise ops), and a
chunked W2 stage (PE psum writes must be 32-aligned in the partition dim).
Host code only restages layouts, gathers the triangle, and takes the top-2
(argsort tie-breaking identical to jax.lax.top_k).
"""

import sys

sys.path.insert(0, "/opt/trn_rl_repo")


import contextlib

import numpy as np

import concourse.bass as bass
import concourse.mybir as mybir
import concourse.tile as tile
from concourse.ap import AP

F32 = mybir.dt.float32
F16 = mybir.dt.float16
U16 = mybir.dt.uint16
ALU = mybir.AluOpType
ACTF = mybir.ActivationFunctionType

B, N, D, H, K = 16, 256, 64, 64, 2
NB = 2              # batches per core
W = 130             # window cols per pair-block (129 needed, 130 for even)
OTW_COLS = 388      # 1 pad col + 386 wrapped node cols, rounded even
NT = N // 2         # 128 pair-blocks per batch
HT = NT // 2        # 64 pair-blocks per half
HCAT = HT * W       # 8320 = 65 * 128
NCHUNK = HCAT // 128  # 65


def build_nc(d2_engine=None, relu_split=None, p2_pool=None):
    """Engine assignment is greedy per piece: d2/p2 go to DVE (94 ns,
    fused subtract+abs_max in 4x mode) or Pool (108 ns); relu+bias pieces
    go to DVE (1x: psum f32 in) or ACT (Relu only - function switches pay
    a 1283 ns activation-table load, so ACT never runs Abs/Copy here).
    Legacy args are ignored."""
    busy = {"dve": 0.0, "act": 0.0, "pool": 0.0}

    def pick(costs):
        e = min(costs, key=lambda k: busy[k] + costs[k])
        busy[e] += costs[e]
        return e

    nc = bass.Bass()
    otw = nc.dram_tensor("otw", [NB, 128, OTW_COLS], F16, kind="ExternalInput")
    icd = nc.dram_tensor("ic", [NB, 128, NT], F16, kind="ExternalInput")
    icd32 = nc.dram_tensor("ic32", [NB, 128, NT], F32, kind="ExternalInput")
    icdn32 = nc.dram_tensor("icn32", [NB, 128, NT], F32, kind="ExternalInput")
    w1a = nc.dram_tensor("w1a", [128, 128], F16, kind="ExternalInput")
    w1b = nc.dram_tensor("w1b", [128, 128], F16, kind="ExternalInput")
    w1c = nc.dram_tensor("w1c", [128, 128], F16, kind="ExternalInput")
    w1d = nc.dram_tensor("w1d", [128, 128], F16, kind="ExternalInput")
    w2 = nc.dram_tensor("w2", [128, 2], F16, kind="ExternalInput")
    b1st = nc.dram_tensor("b1st", [128, 1], F32, kind="ExternalInput")
    qg = nc.dram_tensor("qg", [NB, 2, 128, 2 * NCHUNK], F32,
                        kind="ExternalOutput")

    with contextlib.ExitStack() as ctx:
        tc = ctx.enter_context(tile.TileContext(nc))
        cpool = ctx.enter_context(tc.tile_pool(name="const", bufs=1))
        bpool = ctx.enter_context(tc.tile_pool(name="perb", bufs=2))
        dpool = ctx.enter_context(tc.tile_pool(name="dp", bufs=16))
        hcpool = ctx.enter_context(tc.tile_pool(name="hcat", bufs=4))
        hps = ctx.enter_context(tc.tile_pool(name="hps", bufs=7, space="PSUM"))
        qps = ctx.enter_context(tc.tile_pool(name="qps", bufs=1, space="PSUM"))

        otw_bt = []
        ic_bt = []
        ic32_bt = []
        icn32_bt = []
        for bi in range(NB):
            ot = bpool.tile([128, OTW_COLS], F16, tag=f"otw{bi}")
            icb = bpool.tile([128, NT], F16, tag=f"ic{bi}")
            ic3 = bpool.tile([128, NT], F32, tag=f"ic32{bi}")
            icn = bpool.tile([128, NT], F32, tag=f"icn32{bi}")
            nc.sync.dma_start(ot[:], otw[bi])
            nc.sync.dma_start(icb[:], icd[bi])
            nc.sync.dma_start(ic3[:], icd32[bi])
            nc.sync.dma_start(icn[:], icdn32[bi])
            otw_bt.append(ot)
            ic_bt.append(icb)
            ic32_bt.append(ic3)
            icn32_bt.append(icn)
        w1a_t = cpool.tile([128, 128], F16, tag="w1a")
        w1b_t = cpool.tile([128, 128], F16, tag="w1b")
        w1c_t = cpool.tile([128, 128], F16, tag="w1c")
        w1d_t = cpool.tile([128, 128], F16, tag="w1d")
        w2_t = cpool.tile([128, 2], F16, tag="w2")
        b1_t = cpool.tile([128, 1], F32, tag="b1")
        for t_, d_ in ((w1a_t, w1a), (w1b_t, w1b), (w1c_t, w1c),
                       (w1d_t, w1d), (w2_t, w2), (b1_t, b1st)):
            nc.sync.dma_start(t_[:], d_[:])

        for bi in range(NB):
            otw_b = otw_bt[bi]
            ic_b = ic_bt[bi]
            ic32_b = ic32_bt[bi]
            icn32_b = icn32_bt[bi]

            # s1a/s1b: per-pair-block bias columns (both i-halves stacked)
            ps1 = hps.tile([128, NT], F32, tag="php")
            nc.tensor.matmul(ps1[:], w1a_t[:], ic_b[:], start=True, stop=True)
            s1a = bpool.tile([128, NT], F32, tag="s1a")
            busy["dve"] += 258.0
            nc.vector.tensor_scalar(
                out=s1a[:], in0=ps1[:], scalar1=b1_t[:], scalar2=None,
                op0=ALU.add)
            ps1b = hps.tile([128, NT], F32, tag="php")
            nc.tensor.matmul(ps1b[:], w1b_t[:], ic_b[:], start=True, stop=True)
            s1b = bpool.tile([128, NT], F32, tag="s1b")
            busy["act"] += 292.0
            nc.scalar.activation(s1b[:], ps1b[:], ACTF.Identity, bias=b1_t[:])

            qo = qps.tile([128, 4 * NCHUNK], F32, tag="qo")
            for hf in range(2):
                hcat = hcpool.tile([128, HCAT], F16, tag="hcat")
                for sh in range(HT // 2):
                    s = hf * (HT // 2) + sh
                    t0 = 2 * s
                    dpair = dpool.tile([128, 2 * W], F16, tag="dpair")
                    ppair = dpool.tile([128, 2 * W], F16, tag="ppair")
                    dsub = dpool.tile([128, 2 * W], F16, tag="dsub")
                    for k in range(2):
                        t = t0 + k
                        win = otw_b[:, 2 * t + 2: 2 * t + 2 + W]
                        icol = ic32_b[:, t: t + 1]
                        icoln = icn32_b[:, t: t + 1]
                        dslice = dpair[:, k * W: (k + 1) * W]
                        # d2 = |win - icol|: ACT Abs(win - icol) in one op,
                        # or subtract + u16 sign-clear on DVE/Pool; score by
                        # resulting makespan, not local cost
                        dvars = {
                            "act": (("act", 293.0),),
                            "dve2": (("dve", 94.0), ("dve", 94.0)),
                            "mixPD": (("pool", 108.0), ("dve", 94.0)),
                        }

                        def score(parts):
                            b = dict(busy)
                            for e, c in parts:
                                b[e] += c
                            return max(b.values())

                        dvar = min(dvars, key=lambda v: score(dvars[v]))
                        for e, c in dvars[dvar]:
                            busy[e] += c
                        if dvar == "act":
                            nc.scalar.activation(dslice, win, ACTF.Abs,
                                                 bias=icoln, scale=1.0)
                        else:
                            ds = dsub[:, k * W: (k + 1) * W]
                            if dvar == "mixPD":
                                nc.gpsimd.tensor_tensor(
                                    out=ds, in0=win,
                                    in1=ic_b[:, t: t + 1].broadcast_to(
                                        [128, W]),
                                    op=ALU.subtract)
                            else:
                                nc.vector.tensor_scalar(
                                    out=ds, in0=win, scalar1=icol,
                                    scalar2=None, op0=ALU.subtract)
                            nc.vector.tensor_scalar(
                                out=dslice.bitcast(U16), in0=ds.bitcast(U16),
                                scalar1=0x7fff, scalar2=None,
                                op0=ALU.bitwise_and)
                        pv = {"act": 293.0, "dve": 94.0, "pool": 108.0}
                        pvar = min(pv, key=lambda e: busy[e] + pv[e])
                        busy[pvar] += pv[pvar]
                        pslice = ppair[:, k * W: (k + 1) * W]
                        if pvar == "act":
                            nc.scalar.activation(pslice, win, ACTF.Copy,
                                                 bias=0.0, scale=icol)
                        else:
                            eng = nc.vector if pvar == "dve" else nc.gpsimd
                            eng.tensor_scalar(
                                out=pslice, in0=win,
                                scalar1=icol, scalar2=None, op0=ALU.mult)

                    php = hps.tile([128, 2 * W], F32, tag="php")
                    nc.tensor.matmul(php[:], w1c_t[:], dpair[:],
                                     start=True, stop=False)
                    nc.tensor.matmul(php[:], w1d_t[:], ppair[:],
                                     start=False, stop=False)
                    if s <= 30:
                        # both windows unwrapped: one strided-AP rhs matmul
                        base = otw_b[:, 0:1]
                        rhs = AP(base.tensor, base.offset + 4 * s + 2,
                                 [[OTW_COLS, 128], [2, 2], [1, W]])
                        nc.tensor.matmul(php[:], w1b_t[:], rhs,
                                         start=False, stop=True)
                    else:
                        for k in range(2):
                            t = t0 + k
                            win = otw_b[:, 2 * t + 2: 2 * t + 2 + W]
                            wu = min(W, 255 - 2 * t)
                            sl = php[:, k * W: k * W + wu]
                            nc.tensor.matmul(sl, w1b_t[:], win[:, :wu],
                                             start=False,
                                             stop=(k == 1 and wu == W))
                            if wu < W:
                                nc.tensor.matmul(
                                    php[:, k * W + wu: (k + 1) * W],
                                    w1a_t[:], win[:, wu:],
                                    start=False, stop=(k == 1))

                    for k in range(2):
                        t = t0 + k
                        th = t - hf * HT
                        wu = min(W, 255 - 2 * t)
                        phs = php[:, k * W: (k + 1) * W]
                        hslice = hcat[:, th * W: (th + 1) * W]
                        for lo, hi, s1x in ((0, wu, s1a), (wu, W, s1b)):
                            if lo >= hi:
                                continue
                            w = hi - lo
                            reng = pick({"dve": 1.0417 * w + 125.0,
                                         "act": 0.8333 * w + 185.0})
                            if reng == "act":
                                nc.scalar.activation(
                                    hslice[:, lo:hi], phs[:, lo:hi],
                                    ACTF.Relu, bias=s1x[:, t: t + 1])
                            else:
                                nc.vector.tensor_scalar(
                                    out=hslice[:, lo:hi], in0=phs[:, lo:hi],
                                    scalar1=s1x[:, t: t + 1],
                                    scalar2=0.0, op0=ALU.add, op1=ALU.max)

                qbase = hf * 2 * NCHUNK
                for c in range(NCHUNK):
                    nc.tensor.matmul(
                        qo[:, qbase + 2 * c: qbase + 2 * c + 2],
                        hcat[:, 128 * c: 128 * (c + 1)],
                        w2_t[:], start=True, stop=True)
                qsb = bpool.tile([128, 2 * NCHUNK], F32, tag="qsb")
                qe = pick({"dve": 260.0, "act": 293.0})
                if qe == "act":
                    nc.scalar.activation(
                        qsb[:], qo[:, qbase: qbase + 2 * NCHUNK],
                        ACTF.Copy, bias=0.0)
                else:
                    nc.vector.tensor_copy(
                        qsb[:], qo[:, qbase: qbase + 2 * NCHUNK])
                nc.sync.dma_start(qg[bi, hf], qsb[:])
    return split_sync_waits(nc)


def prep_inputs(O, W1, b1, W2):
    """Host-side layout prep. O: (B,N,D) fp32. Returns per-core input maps."""
    O = np.ascontiguousarray(O, dtype=np.float32)
    W1 = np.asarray(W1, dtype=np.float32)
    b1 = np.asarray(b1, dtype=np.float32)
    W2 = np.asarray(W2, dtype=np.float32)

    def bd(M):  # blockdiag(M, M) -> [128, 128] fp16
        Z = np.zeros((128, 128), np.float16)
        Z[:64, :64] = M.astype(np.float16)
        Z[64:, 64:] = M.astype(np.float16)
        return Z

    w1a, w1b, w1c, w1d = (W1[i * D:(i + 1) * D] for i in range(4))
    w2bd = np.zeros((128, 2), np.float16)
    w2bd[:64, 0] = W2[:, 0].astype(np.float16)
    w2bd[64:, 1] = W2[:, 0].astype(np.float16)
    b1stk = np.concatenate([b1, b1]).reshape(128, 1).astype(np.float32)

    common = {
        "w1a": bd(w1a), "w1b": bd(w1b), "w1c": bd(w1c), "w1d": bd(w1d),
        "w2": w2bd, "b1st": b1stk,
    }

    in_maps = []
    for c in range(8):
        otw = np.zeros((NB, 128, OTW_COLS), np.float16)
        ic = np.empty((NB, 128, NT), np.float16)
        for k in range(NB):
            b = NB * c + k
            OT = O[b].T.astype(np.float16)  # [64, 256]
            wrap = np.concatenate([OT, OT[:, :OTW_COLS - 257]], axis=1)
            otw[k][:64, 1:] = wrap
            otw[k][64:, 1:] = wrap
            ic[k][:64] = O[b][0::2].T.astype(np.float16)
            ic[k][64:] = O[b][1::2].T.astype(np.float16)
        ic32 = ic.astype(np.float32)
        in_maps.append({**common, "otw": otw, "ic": ic,
                        "ic32": ic32, "icn32": -ic32})
    return in_maps


def emulate_core(in_map):
    """Numpy emulation of the device program for one core -> qg array."""
    qg = np.zeros((NB, 2, 128, 2 * NCHUNK), np.float32)
    for k in range(NB):
        otw = in_map["otw"][k].astype(np.float32)
        ic = in_map["ic"][k].astype(np.float32)
        w1a = in_map["w1a"].astype(np.float32)
        w1b = in_map["w1b"].astype(np.float32)
        w1c = in_map["w1c"].astype(np.float32)
        w1d = in_map["w1d"].astype(np.float32)
        w2v = in_map["w2"].astype(np.float32)
        s1a = w1a.T @ ic + in_map["b1st"]
        s1b = w1b.T @ ic + in_map["b1st"]
        for hf in range(2):
            hcat = np.zeros((128, HCAT), np.float32)
            for th in range(HT):
                t = hf * HT + th
                win = otw[:, 2 * t + 2: 2 * t + 2 + W]
                icol = ic[:, t: t + 1]
                wu = min(W, 255 - 2 * t)
                d2 = np.abs(win - icol).astype(np.float16).astype(np.float32)
                p2 = (win * icol).astype(np.float16).astype(np.float32)
                ph = w1c.T @ d2 + w1d.T @ p2
                ph[:, :wu] += w1b.T @ win[:, :wu]
                ph[:, wu:] += w1a.T @ win[:, wu:]
                hs = np.empty_like(ph)
                hs[:, :wu] = np.maximum(ph[:, :wu] + s1a[:, t: t + 1], 0)
                hs[:, wu:] = np.maximum(ph[:, wu:] + s1b[:, t: t + 1], 0)
                hcat[:, th * W: (th + 1) * W] = (
                    hs.astype(np.float16).astype(np.float32))
            for c in range(NCHUNK):
                qg[k, hf][:, 2 * c: 2 * c + 2] = (
                    hcat[:, 128 * c: 128 * (c + 1)].T @ w2v)
    return qg


_GATHER = None


def gather_indices():
    """(half, part, col) per triu pair m for the qg layout; cached."""
    global _GATHER
    if _GATHER is None:
        ii, jj = np.triu_indices(N, 1)
        delta = jj - ii
        r = np.where(delta <= 128, ii, jj)
        cabs = np.where(delta <= 128, jj, ii + 256)
        tg = r // 2
        jw = cabs - (2 * tg + 1)
        assert jw.min() >= 0 and jw.max() < W
        g = (tg % HT) * W + jw
        _GATHER = (r // 128, g % 128, 2 * (g // 128) + (r % 2))
    return _GATHER


def assemble_q(core_outs, b2):
    """core_outs: list of 8 per-core dicts with 'qg'. -> Q (B, M)."""
    half, part, col = gather_indices()
    Q = np.empty((B, N * (N - 1) // 2), np.float32)
    for c in range(8):
        qgv = core_outs[c]["qg"]  # [NB, 2, 128, 130]
        for k in range(NB):
            Q[NB * c + k] = qgv[k, half, part, col]
    return Q + np.float32(np.asarray(b2).reshape(-1)[0])


def topk_mask(Q):
    idx = np.argsort(-Q, axis=1, kind="stable")[:, :K]
    mask = np.zeros(Q.shape, dtype=bool)
    mask[np.arange(Q.shape[0])[:, None], idx] = True
    return mask


def split_sync_waits(nc, maxw=1):
    """This walrus build rejects >maxw sync waits per instruction: hoist
    excess waits onto same-engine no-ops inserted just before."""
    ctr = [0]
    for f in nc.m.functions:
        for bb in f.blocks:
            out = []
            for ins in bb.instructions:
                si = ins.sync_info
                waits = list(si.on_wait or []) if si else []
                if len(waits) > maxw:
                    si.on_wait = waits[-maxw:]
                    rest = waits[:-maxw]
                    for i in range(0, len(rest), maxw):
                        nop = mybir.InstNoOp(name=f"I-wsplit-{ctr[0]}")
                        ctr[0] += 1
                        nop.engine = ins.engine
                        nop.sync_info = mybir.SyncInfo(
                            on_wait=rest[i:i + maxw], on_update=[])
                        nc.register_instruction(nop, overwrite=True)
                        out.append(nop)
                out.append(ins)
            if ctr[0]:
                bb.instructions = out
    return nc


_NC = None


def kernel(O, W1, b1, W2, b2):
    global _NC
    if _NC is None:
        _NC = build_nc()
    in_maps = prep_inputs(O, W1, b1, W2)
    from concourse.bass_utils import run_bass_kernel_spmd
    res = run_bass_kernel_spmd(_NC, in_maps, list(range(8)))
    Q = assemble_q(res.results, b2)
    return Q, topk_mask(Q)

